# revision 1
# baseline (speedup 1.0000x reference)
"""Trainium2 Bass kernel for rank-1 attention + linear (nn_Attention).

Reference computation (S=256, B=128, D=4096):
    scores   = einsum('sbd,bd->bs', inp, hidden[0])      # dot each enc state with hidden
    attn     = softmax(scores, axis=1)                   # over S
    weighted = einsum('bs,sbd->bd', attn, inp)
    concat   = [weighted, hidden[0]]   # [B, 2D]
    out      = concat @ W.T + b        # [1, B, D]

Distribution over 8 NeuronCores:
  - attention part: data-parallel over B (16 batches per core)
  - linear part: W sharded over output dim (512 rows per core); weighted
    vectors exchanged with an on-chip AllGather.

Per-core dataflow:
  scores  : DVE fused tensor_tensor_reduce (mult + free-dim add) against a
            gpsimd partition-broadcast of the hidden row
  softmax : gpsimd partition_all_reduce (max/add) + ACT exp + DVE reciprocal
  weighted: PE matmuls with a column-masked attn matrix (lhsT [s,16], only
            col b nonzero) accumulating all 16 batches into one PSUM region
  linear  : host-pretransposed W (f-major) + on-chip PE transpose of the
            allgathered weighted matrix; 64 accumulating matmuls
"""

import sys

if "/opt/trn_rl_repo" not in sys.path:
    sys.path.insert(0, "/opt/trn_rl_repo")

import numpy as np


# ----------------------------------------------------------------------------
# Program builder
# ----------------------------------------------------------------------------

def build_program(S=256, B=128, D=4096, n_cores=8, no_collective=False, stage="full",
                  use_f32r=False):
    """Build the SPMD Bass program. Returns finalized nc.

    no_collective=True replaces the AllGather with a local DMA (functionally
    wrong for n_cores>1) so the single-core TimelineSim can model timing.
    stage: "attn" stops after the weighted sums (debug), "attn_ag" adds the
    exchange (debug), "full" is the real kernel.
    """
    import concourse.bacc as bacc
    import concourse.bass_isa as bass_isa
    import concourse.mybir as mybir
    import concourse.tile as tile
    from concourse import library_config

    f32 = mybir.dt.float32
    f32r = mybir.dt.float32r if use_f32r else mybir.dt.float32
    P = 128
    Bc = B // n_cores                 # batches per core
    ST = S // P                       # s-tiles per batch (2)
    F = 2 * D                         # concat feature dim (8192)
    DOUT = D // n_cores               # output-dim shard per core (512)
    NC_D = D // 512                   # 512-wide d-chunks for weighted MMs (8)
    NT_W = D // P                     # 128-wide transpose chunks of weighted (32)
    NT_H = D // P                     # 128-wide chunks of hidden (32)
    NKF = F // P                      # 128-wide k-chunks of the linear (64)

    nc = bacc.Bacc(None, target_bir_lowering=False)

    inp = nc.dram_tensor("inp", [S, Bc, D], f32, kind="ExternalInput")
    hid = nc.dram_tensor("hid", [Bc, D], f32, kind="ExternalInput")
    hidT = nc.dram_tensor("hidT", [D, B], f32, kind="ExternalInput")
    wt = nc.dram_tensor("wt", [F, DOUT], f32, kind="ExternalInput")
    bias = nc.dram_tensor("bias", [1, DOUT], f32, kind="ExternalInput")
    ident = nc.dram_tensor("ident", [P, P], f32, kind="ExternalInput")
    out = nc.dram_tensor("out", [B, DOUT], f32, kind="ExternalOutput")

    cc_in = nc.dram_tensor("cc_in", [Bc, D], f32)
    cc_out = nc.dram_tensor("cc_out", [B, D], f32, addr_space="Shared")

    if stage == "nop":
        with tile.TileContext(nc) as tc:
            with tc.tile_pool(name="sb", bufs=1) as sb:
                t0 = sb.tile([P, 512], f32)
                nc.sync.dma_start(out=t0, in_=inp[0:P, 0, 0:512])
                nc.sync.dma_start(out=out[0:P, 0:512], in_=t0)
                t1 = sb.tile([1, 1], f32)
                nc.sync.dma_start(out=t1, in_=hid[0:1, 0:1])
                t2 = sb.tile([1, 1], f32)
                nc.sync.dma_start(out=t2, in_=hidT[0:1, 0:1])
                t3 = sb.tile([1, 1], f32)
                nc.sync.dma_start(out=t3, in_=wt[0:1, 0:1])
                t4 = sb.tile([1, 1], f32)
                nc.sync.dma_start(out=t4, in_=bias[0:1, 0:1])
                t5 = sb.tile([1, 1], f32)
                nc.sync.dma_start(out=t5, in_=ident[0:1, 0:1])
        nc.finalize()
        return nc

    with tile.TileContext(nc) as tc:
        import contextlib

        with contextlib.ExitStack() as ctx:
            persist = ctx.enter_context(tc.tile_pool(name="persist", bufs=1))

            nc.gpsimd.load_library(library_config.attn)

            ident_sb = persist.tile([P, P], f32)
            nc.sync.dma_start(out=ident_sb, in_=ident[:, :])

            # masked attn weights: [s, t, b, col]; col b of slice (t, b) is
            # batch b's attn column, everything else stays zero
            attn_diag = persist.tile([P, ST, Bc, Bc], f32r)
            nc.vector.memset(attn_diag[:, :, :, :].bitcast(f32), 0.0)

            wsum = persist.tile([Bc, D], f32)

            # linear-stage inputs that stream/land during the batch loop
            wtp = ctx.enter_context(tc.tile_pool(name="wtp", bufs=4))
            hT_sb = persist.tile([P, NT_H, B], f32r)
            nc.sync.dma_start(
                out=hT_sb,
                in_=hidT.rearrange("(c p) b -> p c b", p=P).bitcast(f32r),
            )

            # ---------------- attention (batch loop) ----------------
            with contextlib.ExitStack() as loop_ctx:
                natp = loop_ctx.enter_context(tc.tile_pool(name="nat", bufs=5))
                hbp = loop_ctx.enter_context(tc.tile_pool(name="hb", bufs=2))
                prodp = loop_ctx.enter_context(tc.tile_pool(name="prod", bufs=1))
                smalls = loop_ctx.enter_context(tc.tile_pool(name="smalls", bufs=3))
                waccp = loop_ctx.enter_context(
                    tc.tile_pool(name="wacc", bufs=1, space="PSUM")
                )

                wacc = waccp.tile([Bc, D], f32)

                hrow2 = persist.tile([1, 2, D], f32)

                def emit_hb(b):
                    nc.sync.dma_start(
                        out=hrow2[:, b % 2, :],
                        in_=hid.rearrange("(o b) d -> o b d", o=1)[:, b, :],
                    )
                    hb = hbp.tile([P, D], f32, tag="hb")
                    nc.gpsimd.partition_broadcast(hb, hrow2[:, b % 2, :])
                    return hb

                hbs = {0: emit_hb(0)}

                for b in range(Bc):
                    hb = hbs.pop(b)

                    nats = []
                    sc_b = smalls.tile([P, ST], f32, tag="sc")
                    for t in range(ST):
                        nat = natp.tile([P, D], f32r, tag="nat")
                        nc.sync.dma_start(
                            out=nat, in_=inp[t * P : (t + 1) * P, b, :].bitcast(f32r)
                        )
                        nats.append(nat)
                        prod = prodp.tile([P, D], mybir.dt.bfloat16, tag="prod")
                        nc.vector.scalar_tensor_tensor(
                            out=prod,
                            in0=nat[:, :].bitcast(f32),
                            scalar=1.0,
                            in1=hb,
                            op0=mybir.AluOpType.mult,
                            op1=mybir.AluOpType.mult,
                            accum_out=sc_b[:, t : t + 1],
                        )

                    # prefetch next batch's hidden broadcast so the Pool
                    # FIFO doesn't serialize it behind this batch's PARs
                    if b + 1 < Bc:
                        hbs[b + 1] = emit_hb(b + 1)

                    # softmax over s (partition dim x ST columns)
                    mx2 = smalls.tile([P, ST], f32, tag="mx2")
                    nc.gpsimd.partition_all_reduce(
                        mx2, sc_b, channels=P, reduce_op=bass_isa.ReduceOp.max
                    )
                    negm = smalls.tile([P, 1], f32, tag="negm")
                    nc.vector.tensor_reduce(
                        out=negm, in_=mx2, axis=mybir.AxisListType.X,
                        op=mybir.AluOpType.max, negate=True,
                    )
                    e_b = smalls.tile([P, ST], f32, tag="e_b")
                    s1 = smalls.tile([P, 1], f32, tag="s1")
                    nc.scalar.activation(
                        out=e_b,
                        in_=sc_b,
                        func=mybir.ActivationFunctionType.Exp,
                        bias=negm,
                        scale=1.0,
                        accum_out=s1,
                    )
                    sig = smalls.tile([P, 1], f32, tag="sig")
                    nc.gpsimd.partition_all_reduce(
                        sig, s1, channels=P, reduce_op=bass_isa.ReduceOp.add
                    )
                    r = smalls.tile([P, 1], f32, tag="r")
                    nc.vector.reciprocal(r, sig)
                    attn_b = smalls.tile([P, ST], f32, tag="attn_b")
                    nc.scalar.activation(
                        out=attn_b,
                        in_=e_b,
                        func=mybir.ActivationFunctionType.Copy,
                        bias=0.0,
                        scale=r,
                    )
                    # scatter the two attn columns into their diagonal slots
                    for t in range(ST):
                        nc.scalar.activation(
                            out=attn_diag[:, t, b, b : b + 1],
                            in_=attn_b[:, t : t + 1],
                            func=mybir.ActivationFunctionType.Copy,
                        )

                    # weighted sums: accumulate into wacc rows via masked lhsT
                    for t in range(ST):
                        for c in range(NC_D):
                            nc.tensor.matmul(
                                wacc[:, c * 512 : (c + 1) * 512],
                                attn_diag[:, t, b, :],
                                nats[t][:, c * 512 : (c + 1) * 512],
                                start=(b == 0 and t == 0),
                                stop=(b == Bc - 1 and t == ST - 1),
                            )

                # evacuate weighted PSUM
                nc.scalar.activation(
                    out=wsum, in_=wacc, func=mybir.ActivationFunctionType.Copy
                )

            # early Wt prefetch on the SP queue: SP drains its loop DMAs
            # first, so these transfers land while the last batch computes
            wt_early = {}
            if stage == "full":
                for c in range(NKF // 2, NKF // 2 + 4):
                    wt_sb = wtp.tile([P, DOUT], f32r, tag="wt")
                    nc.sync.dma_start(
                        out=wt_sb, in_=wt[c * P : (c + 1) * P, :].bitcast(f32r)
                    )
                    wt_early[c] = wt_sb

            # ---------------- exchange ----------------
            if stage == "attn":
                nc.sync.dma_start(out=out[:Bc, :DOUT], in_=wsum[:, :DOUT])
            else:
                nc.sync.dma_start(out=cc_in[:, :], in_=wsum)
                if no_collective:
                    for k in range(n_cores):
                        nc.sync.dma_start(
                            out=cc_out[k * Bc : (k + 1) * Bc, :], in_=cc_in[:, :]
                        )
                else:
                    nc.gpsimd.collective_compute(
                        "AllGather",
                        mybir.AluOpType.bypass,
                        replica_groups=[list(range(n_cores))],
                        ins=[cc_in[:, :]],
                        outs=[cc_out[:, :]],
                    )
            if stage == "attn_ag":
                wag_dbg = persist.tile([B, DOUT], f32)
                nc.sync.dma_start(out=wag_dbg, in_=cc_out[:, :DOUT])
                nc.sync.dma_start(out=out[:, :], in_=wag_dbg)

            # ---------------- linear ----------------
            if stage != "full":
                lin_enabled = False
            else:
                lin_enabled = True
            if lin_enabled:
              with contextlib.ExitStack() as lin_ctx:
                tailp = lin_ctx.enter_context(tc.tile_pool(name="tail", bufs=1))
                wTp = lin_ctx.enter_context(tc.tile_pool(name="wTp", bufs=NT_W))
                tpp = lin_ctx.enter_context(
                    tc.tile_pool(name="tp", bufs=4, space="PSUM")
                )
                linp = lin_ctx.enter_context(
                    tc.tile_pool(name="lin", bufs=1, space="PSUM")
                )

                out_ps = linp.tile([B, DOUT], f32)

                # stream all Wt chunks on the ACT DGE queue (SP is busy with
                # the exchange DMAs; separate queue avoids head-of-line block).
                # First few chunks prefetch into the small always-live pool
                # during the batch loop; the bulk goes to a big pool that
                # reuses the loop's SBUF so streaming isn't consumption-gated.
                wtbig = lin_ctx.enter_context(tc.tile_pool(name="wtbig", bufs=24))
                wt_tiles = dict(wt_early)
                for c in list(range(NKF // 2 + 4, NKF)) + list(range(NKF // 2)):
                    wt_sb = wtbig.tile([P, DOUT], f32r, tag="wt")
                    nc.scalar.dma_start(
                        out=wt_sb, in_=wt[c * P : (c + 1) * P, :].bitcast(f32r)
                    )
                    wt_tiles[c] = wt_sb

                # hidden half first: lhsT chunks come straight from hidT input
                for c in range(NKF // 2, NKF):
                    nc.tensor.matmul(
                        out_ps,
                        hT_sb[:, c - NKF // 2, :],
                        wt_tiles.pop(c),
                        start=(c == NKF // 2),
                        stop=False,
                    )

                # weighted half: gather result, transpose on PE, then matmul
                wag = tailp.tile([B, D], f32)
                nc.sync.dma_start(out=wag, in_=cc_out[:, :])

                wTs = []
                for c in range(NT_W):
                    tp_ps = tpp.tile([P, B], f32, tag="tp")
                    nc.tensor.transpose(
                        tp_ps, wag[:, c * P : (c + 1) * P], ident_sb[:B, :B]
                    )
                    wT = wTp.tile([P, B], f32r, tag="wT")
                    nc.vector.tensor_copy(wT, tp_ps)
                    wTs.append(wT)

                for c in range(NT_W):
                    nc.tensor.matmul(
                        out_ps,
                        wTs[c],
                        wt_tiles.pop(c),
                        start=False,
                        stop=(c == NT_W - 1),
                    )

                # bias add + store
                bias_sb = tailp.tile([1, DOUT], f32)
                nc.sync.dma_start(out=bias_sb, in_=bias[:, :])
                bias_bc = tailp.tile([B, DOUT], f32)
                nc.gpsimd.partition_broadcast(bias_bc, bias_sb)
                out_sb = tailp.tile([B, DOUT], f32)
                nc.vector.tensor_add(out_sb, out_ps, bias_bc)
                nc.sync.dma_start(out=out[:, :], in_=out_sb)

    nc.finalize()
    return nc


_CACHE = {}


def _get_program(S, B, D, n_cores):
    key = (S, B, D, n_cores)
    if key not in _CACHE:
        _CACHE[key] = build_program(S, B, D, n_cores)
    return _CACHE[key]


def make_in_maps(inp, hidden, W, b, n_cores=8):
    """Shard host inputs into per-core input maps."""
    S, B, D = inp.shape
    Bc = B // n_cores
    DOUT = W.shape[0] // n_cores
    hidT = np.ascontiguousarray(hidden[0].T)          # [D, B]
    ident = np.eye(128, dtype=np.float32)
    in_maps = []
    for k in range(n_cores):
        in_maps.append(
            {
                "inp": np.ascontiguousarray(inp[:, k * Bc : (k + 1) * Bc, :]),
                "hid": np.ascontiguousarray(hidden[0, k * Bc : (k + 1) * Bc, :]),
                "hidT": hidT,
                "wt": np.ascontiguousarray(W[k * DOUT : (k + 1) * DOUT, :].T),
                "bias": np.ascontiguousarray(
                    b[k * DOUT : (k + 1) * DOUT].reshape(1, DOUT)
                ),
                "ident": ident,
            }
        )
    return in_maps


def kernel(inp, hidden, W, b, trace=False):
    from concourse.bass_utils import run_bass_kernel_spmd

    inp = np.asarray(inp, dtype=np.float32)
    hidden = np.asarray(hidden, dtype=np.float32)
    W = np.asarray(W, dtype=np.float32)
    b = np.asarray(b, dtype=np.float32)

    S, B, D = inp.shape
    n_cores = 8
    nc = _get_program(S, B, D, n_cores)
    in_maps = make_in_maps(inp, hidden, W, b, n_cores)
    res = run_bass_kernel_spmd(nc, in_maps, core_ids=list(range(n_cores)))
    outs = [res.results[k]["out"] for k in range(n_cores)]
    full = np.concatenate(outs, axis=1)  # [B, D]
    if trace:
        return full[None, :, :], res
    return full[None, :, :]



# revision 4
# speedup vs baseline: 1.0868x; 1.0868x over previous
"""Trainium2 Bass kernel for rank-1 attention + linear (nn_Attention).

Reference computation (S=256, B=128, D=4096):
    scores   = einsum('sbd,bd->bs', inp, hidden[0])      # dot each enc state with hidden
    attn     = softmax(scores, axis=1)                   # over S
    weighted = einsum('bs,sbd->bd', attn, inp)
    concat   = [weighted, hidden[0]]   # [B, 2D]
    out      = concat @ W.T + b        # [1, B, D]

Distribution over 8 NeuronCores:
  - attention part: data-parallel over B (16 batches per core)
  - linear part: W sharded over output dim (512 rows per core); weighted
    vectors exchanged with an on-chip AllGather.

All heavy tensors (inp, hidden, W) are downcast to bf16 on the host:
halves HBM traffic and runs PE matmuls / DVE products at full rate
(f32 matmuls cost 4 cycles/row on trn2 PE; bf16 costs 1).

Per-core dataflow:
  scores  : DVE fused scalar_tensor_tensor (mult + free-dim accum) against a
            gpsimd partition-broadcast of the hidden row
  softmax : gpsimd partition_all_reduce (max/add) + ACT exp + DVE reciprocal
  weighted: PE matmuls with a column-masked attn matrix (lhsT [s,16], only
            col b nonzero) accumulating all 16 batches into one PSUM region
  linear  : host-pretransposed W (f-major) + on-chip PE transpose of the
            allgathered weighted matrix; 64 accumulating matmuls
"""

import sys

if "/opt/trn_rl_repo" not in sys.path:
    sys.path.insert(0, "/opt/trn_rl_repo")

import numpy as np

# "bf16": host downcasts inp/hid/hidT/wt/ident to bf16 (fast PE + half DMA)
# "f32r": full-precision data, matmuls in f32r (1 cyc/row at >=256 moving)
# "f32" : baseline precision and speed
MODE = "fp16"


# ----------------------------------------------------------------------------
# Program builder
# ----------------------------------------------------------------------------

def build_program(S=256, B=128, D=4096, n_cores=8, no_collective=False,
                  mode=MODE):
    """Build the SPMD Bass program. Returns finalized nc."""
    import concourse.bacc as bacc
    import concourse.bass_isa as bass_isa
    import concourse.mybir as mybir
    import concourse.tile as tile
    from concourse import library_config

    f32 = mybir.dt.float32
    bf16 = mybir.dt.bfloat16
    if mode == "fp16":
        dlo = mybir.dt.float16
        tlo = mybir.dt.float16
    elif mode == "bf16":
        dlo = bf16          # dram dtype of downcast inputs
        tlo = bf16          # SBUF tile dtype for matmul/product operands
    elif mode == "f32r":
        dlo = f32
        tlo = mybir.dt.float32r
    else:
        dlo = f32
        tlo = f32

    def mmin(ap):
        # view a dram AP as the matmul-operand dtype
        return ap if dlo == tlo else ap.bitcast(tlo)

    def vin(ap):
        # view a tile AP as a DVE-operand dtype
        return ap if tlo != mybir.dt.float32r else ap.bitcast(f32)

    P = 128
    Bc = B // n_cores                 # batches per core (16)
    ST = S // P                       # s-tiles per batch (2)
    F = 2 * D                         # concat feature dim (8192)
    DOUT = D // n_cores               # output-dim shard per core (512)
    NC_D = D // 512                   # 512-wide d-chunks for weighted MMs (8)
    NT_W = D // P                     # 128-wide transpose chunks of weighted (32)
    NT_H = D // P                     # 128-wide chunks of hidden (32)
    NKF = F // P                      # 128-wide k-chunks of the linear (64)

    nc = bacc.Bacc(None, target_bir_lowering=False)

    inp = nc.dram_tensor("inp", [S, Bc, D], dlo, kind="ExternalInput")
    hid = nc.dram_tensor("hid", [Bc, D], dlo, kind="ExternalInput")
    hidT = nc.dram_tensor("hidT", [D, B], dlo, kind="ExternalInput")
    wt = nc.dram_tensor("wt", [F, DOUT], dlo, kind="ExternalInput")
    bias = nc.dram_tensor("bias", [1, DOUT], f32, kind="ExternalInput")
    ident = nc.dram_tensor("ident", [P, P], dlo, kind="ExternalInput")
    out = nc.dram_tensor("out", [B, DOUT], f32, kind="ExternalOutput")

    cc_in = nc.dram_tensor("cc_in", [Bc, D], dlo)
    cc_out = nc.dram_tensor("cc_out", [B, D], dlo, addr_space="Shared")

    with tile.TileContext(nc) as tc:
        import contextlib

        with contextlib.ExitStack() as ctx:
            persist = ctx.enter_context(tc.tile_pool(name="persist", bufs=1))

            nc.gpsimd.load_library(library_config.attn)

            ident_sb = persist.tile([P, P], tlo)
            nc.scalar.dma_start(out=ident_sb, in_=mmin(ident[:, :]))

            # masked attn weights: [s, t, b, col]; col b of slice (t, b) is
            # batch b's attn column, everything else stays zero
            attn_diag = persist.tile([P, ST, Bc, Bc], tlo)
            nc.vector.memset(attn_diag[:, :, :, :], 0.0)

            wsum = persist.tile([Bc, D], dlo if mode in ("bf16", "fp16") else f32)

            # linear-stage inputs that stream/land during the batch loop:
            # hidden^T chunks (lhsT of the hidden half) and both W halves,
            # each as one big DMA on queues other than the nat-tile queue.
            hT_sb = persist.tile([P, NT_H, B], tlo)
            nc.scalar.dma_start(
                out=hT_sb,
                in_=mmin(hidT.rearrange("(c p) b -> p c b", p=P)),
            )
            wt_hi = persist.tile([P, NKF // 2, DOUT], tlo)
            nc.scalar.dma_start(
                out=wt_hi,
                in_=mmin(
                    wt[(NKF // 2) * P:, :].rearrange("(c p) d -> p c d", p=P)
                ),
            )
            wt_lo = persist.tile([P, NKF // 2, DOUT], tlo)
            nc.scalar.dma_start(
                out=wt_lo,
                in_=mmin(
                    wt[: (NKF // 2) * P, :].rearrange("(c p) d -> p c d", p=P)
                ),
            )

            # ---------------- attention (batch loop) ----------------
            with contextlib.ExitStack() as loop_ctx:
                natp = loop_ctx.enter_context(tc.tile_pool(name="nat", bufs=5))
                hbp = loop_ctx.enter_context(tc.tile_pool(name="hb", bufs=2))
                prodp = loop_ctx.enter_context(tc.tile_pool(name="prod", bufs=1))
                smalls = loop_ctx.enter_context(tc.tile_pool(name="smalls", bufs=3))
                waccp = loop_ctx.enter_context(
                    tc.tile_pool(name="wacc", bufs=1, space="PSUM")
                )

                wacc = waccp.tile([Bc, D], f32)

                hrow2 = persist.tile([1, 2, D], dlo)

                def emit_hb(b):
                    nc.sync.dma_start(
                        out=hrow2[:, b % 2, :],
                        in_=hid.rearrange("(o b) d -> o b d", o=1)[:, b, :],
                    )
                    hb = hbp.tile([P, D], dlo, tag="hb")
                    nc.gpsimd.partition_broadcast(hb, hrow2[:, b % 2, :])
                    return hb

                hbs = {0: emit_hb(0)}

                for b in range(Bc):
                    hb = hbs.pop(b)

                    nats = []
                    sc_b = smalls.tile([P, ST], f32, tag="sc")
                    for t in range(ST):
                        nat = natp.tile([P, D], tlo, tag="nat")
                        nc.sync.dma_start(
                            out=nat, in_=mmin(inp[t * P : (t + 1) * P, b, :])
                        )
                        nats.append(nat)
                        prod = prodp.tile([P, D], dlo if mode in ("bf16", "fp16") else mybir.dt.bfloat16, tag="prod")
                        nc.vector.scalar_tensor_tensor(
                            out=prod,
                            in0=vin(nat[:, :]),
                            scalar=1.0,
                            in1=vin(hb[:, :]),
                            op0=mybir.AluOpType.mult,
                            op1=mybir.AluOpType.mult,
                            accum_out=sc_b[:, t : t + 1],
                        )

                    # prefetch next batch's hidden broadcast so the Pool
                    # FIFO doesn't serialize it behind this batch's PARs
                    if b + 1 < Bc:
                        hbs[b + 1] = emit_hb(b + 1)

                    # softmax over s (partition dim x ST columns)
                    mx2 = smalls.tile([P, ST], f32, tag="mx2")
                    nc.gpsimd.partition_all_reduce(
                        mx2, sc_b, channels=P, reduce_op=bass_isa.ReduceOp.max
                    )
                    negm = smalls.tile([P, 1], f32, tag="negm")
                    nc.vector.tensor_reduce(
                        out=negm, in_=mx2, axis=mybir.AxisListType.X,
                        op=mybir.AluOpType.max, negate=True,
                    )
                    e_b = smalls.tile([P, ST], f32, tag="e_b")
                    s1 = smalls.tile([P, 1], f32, tag="s1")
                    nc.scalar.activation(
                        out=e_b,
                        in_=sc_b,
                        func=mybir.ActivationFunctionType.Exp,
                        bias=negm,
                        scale=1.0,
                        accum_out=s1,
                    )
                    sig = smalls.tile([P, 1], f32, tag="sig")
                    nc.gpsimd.partition_all_reduce(
                        sig, s1, channels=P, reduce_op=bass_isa.ReduceOp.add
                    )
                    r = smalls.tile([P, 1], f32, tag="r")
                    nc.vector.reciprocal(r, sig)
                    attn_b = smalls.tile([P, ST], f32, tag="attn_b")
                    nc.scalar.activation(
                        out=attn_b,
                        in_=e_b,
                        func=mybir.ActivationFunctionType.Copy,
                        bias=0.0,
                        scale=r,
                    )
                    # scatter the two attn columns into their diagonal slots
                    for t in range(ST):
                        nc.scalar.activation(
                            out=attn_diag[:, t, b, b : b + 1],
                            in_=attn_b[:, t : t + 1],
                            func=mybir.ActivationFunctionType.Copy,
                        )

                    # weighted sums: accumulate into wacc rows via masked lhsT
                    for t in range(ST):
                        for c in range(NC_D):
                            nc.tensor.matmul(
                                wacc[:, c * 512 : (c + 1) * 512],
                                attn_diag[:, t, b, :],
                                nats[t][:, c * 512 : (c + 1) * 512],
                                start=(b == 0 and t == 0),
                                stop=(b == Bc - 1 and t == ST - 1),
                            )

                # evacuate weighted PSUM (converts to bf16 in bf16 mode)
                nc.scalar.activation(
                    out=wsum, in_=wacc, func=mybir.ActivationFunctionType.Copy
                )

            # ---------------- exchange ----------------
            nc.sync.dma_start(out=cc_in[:, :], in_=wsum)
            if no_collective:
                for k in range(n_cores):
                    nc.sync.dma_start(
                        out=cc_out[k * Bc : (k + 1) * Bc, :], in_=cc_in[:, :]
                    )
            else:
                nc.gpsimd.collective_compute(
                    "AllGather",
                    mybir.AluOpType.bypass,
                    replica_groups=[list(range(n_cores))],
                    ins=[cc_in[:, :]],
                    outs=[cc_out[:, :]],
                )

            # ---------------- linear ----------------
            with contextlib.ExitStack() as lin_ctx:
                tailp = lin_ctx.enter_context(tc.tile_pool(name="tail", bufs=1))
                wTp = lin_ctx.enter_context(tc.tile_pool(name="wTp", bufs=NT_W))
                tpp = lin_ctx.enter_context(
                    tc.tile_pool(name="tp", bufs=4, space="PSUM")
                )
                linp = lin_ctx.enter_context(
                    tc.tile_pool(name="lin", bufs=1, space="PSUM")
                )

                out_ps = linp.tile([B, DOUT], f32)

                # hidden half first: lhsT chunks come straight from hidT input
                for c in range(NKF // 2):
                    nc.tensor.matmul(
                        out_ps,
                        hT_sb[:, c, :],
                        wt_hi[:, c, :],
                        start=(c == 0),
                        stop=False,
                    )

                # weighted half: gather result, transpose on PE, then matmul
                wag = tailp.tile([B, D], dlo if mode in ("bf16", "fp16") else f32)
                nc.sync.dma_start(out=wag, in_=cc_out[:, :])

                wTs = []
                for c in range(NT_W):
                    tp_ps = tpp.tile([P, B], tlo, tag="tp")
                    nc.tensor.transpose(
                        tp_ps, mmin(wag[:, c * P : (c + 1) * P]),
                        ident_sb[:B, :B],
                    )
                    wT = wTp.tile([P, B], tlo, tag="wT")
                    nc.vector.tensor_copy(wT, tp_ps)
                    wTs.append(wT)

                for c in range(NT_W):
                    nc.tensor.matmul(
                        out_ps,
                        wTs[c],
                        wt_lo[:, c, :],
                        start=False,
                        stop=(c == NT_W - 1),
                    )

                # bias add + store
                bias_sb = tailp.tile([1, DOUT], f32)
                nc.scalar.dma_start(out=bias_sb, in_=bias[:, :])
                bias_bc = tailp.tile([B, DOUT], f32)
                nc.gpsimd.partition_broadcast(bias_bc, bias_sb)
                out_sb = tailp.tile([B, DOUT], f32)
                nc.vector.tensor_add(out_sb, out_ps, bias_bc)
                nc.sync.dma_start(out=out[:, :], in_=out_sb)

    nc.finalize()
    return nc


_CACHE = {}


def _get_program(S, B, D, n_cores):
    key = (S, B, D, n_cores)
    if key not in _CACHE:
        _CACHE[key] = build_program(S, B, D, n_cores)
    return _CACHE[key]


def make_in_maps(inp, hidden, W, b, n_cores=8):
    """Shard host inputs into per-core input maps."""
    S, B, D = inp.shape
    Bc = B // n_cores
    DOUT = W.shape[0] // n_cores
    if MODE == "fp16":
        lo = np.float16
    elif MODE == "bf16":
        import ml_dtypes

        lo = ml_dtypes.bfloat16
    else:
        lo = np.float32
    inp = np.asarray(inp, dtype=np.float32).astype(lo)
    hid0 = np.asarray(hidden[0], dtype=np.float32).astype(lo)   # [B, D]
    hidT = np.ascontiguousarray(hid0.T)                         # [D, B]
    wtT = np.asarray(W, dtype=np.float32).T.astype(lo)          # [F, 4096]
    ident = np.eye(128, dtype=np.float32).astype(lo)
    in_maps = []
    for k in range(n_cores):
        in_maps.append(
            {
                "inp": np.ascontiguousarray(inp[:, k * Bc : (k + 1) * Bc, :]),
                "hid": np.ascontiguousarray(hid0[k * Bc : (k + 1) * Bc, :]),
                "hidT": hidT,
                "wt": np.ascontiguousarray(wtT[:, k * DOUT : (k + 1) * DOUT]),
                "bias": np.ascontiguousarray(
                    np.asarray(b[k * DOUT : (k + 1) * DOUT], dtype=np.float32)
                    .reshape(1, DOUT)
                ),
                "ident": ident,
            }
        )
    return in_maps


def kernel(inp, hidden, W, b, trace=False):
    from concourse.bass_utils import run_bass_kernel_spmd

    inp = np.asarray(inp, dtype=np.float32)
    hidden = np.asarray(hidden, dtype=np.float32)
    W = np.asarray(W, dtype=np.float32)
    b = np.asarray(b, dtype=np.float32)

    S, B, D = inp.shape
    n_cores = 8
    nc = _get_program(S, B, D, n_cores)
    in_maps = make_in_maps(inp, hidden, W, b, n_cores)
    res = run_bass_kernel_spmd(nc, in_maps, core_ids=list(range(n_cores)))
    outs = [res.results[k]["out"] for k in range(n_cores)]
    full = np.concatenate(outs, axis=1)  # [B, D]
    if trace:
        return full[None, :, :], res
    return full[None, :, :]


# revision 5
# speedup vs baseline: 1.3348x; 1.2281x over previous
"""Trainium2 Bass kernel for rank-1 attention + linear (nn_Attention).

Reference computation (S=256, B=128, D=4096):
    scores   = einsum('sbd,bd->bs', inp, hidden[0])      # dot each enc state with hidden
    attn     = softmax(scores, axis=1)                   # over S
    weighted = einsum('bs,sbd->bd', attn, inp)
    concat   = [weighted, hidden[0]]   # [B, 2D]
    out      = concat @ W.T + b        # [1, B, D]

Distribution over 8 NeuronCores:
  - attention part: data-parallel over B (16 batches per core)
  - linear part: W sharded over output dim (512 rows per core); weighted
    vectors exchanged with an on-chip AllGather.

All heavy tensors (inp, hidden, W) are downcast to bf16 on the host:
halves HBM traffic and runs PE matmuls / DVE products at full rate
(f32 matmuls cost 4 cycles/row on trn2 PE; bf16 costs 1).

Per-core dataflow:
  scores  : DVE fused scalar_tensor_tensor (mult + free-dim accum) against a
            gpsimd partition-broadcast of the hidden row
  softmax : gpsimd partition_all_reduce (max/add) + ACT exp + DVE reciprocal
  weighted: PE matmuls with a column-masked attn matrix (lhsT [s,16], only
            col b nonzero) accumulating all 16 batches into one PSUM region
  linear  : host-pretransposed W (f-major) + on-chip PE transpose of the
            allgathered weighted matrix; 64 accumulating matmuls
"""

import sys

if "/opt/trn_rl_repo" not in sys.path:
    sys.path.insert(0, "/opt/trn_rl_repo")

import numpy as np

# "bf16": host downcasts inp/hid/hidT/wt/ident to bf16 (fast PE + half DMA)
# "f32r": full-precision data, matmuls in f32r (1 cyc/row at >=256 moving)
# "f32" : baseline precision and speed
MODE = "fp16"


# ----------------------------------------------------------------------------
# Program builder
# ----------------------------------------------------------------------------

def build_program(S=256, B=128, D=4096, n_cores=8, no_collective=False,
                  mode=MODE):
    """Build the SPMD Bass program. Returns finalized nc."""
    import concourse.bacc as bacc
    import concourse.bass_isa as bass_isa
    import concourse.mybir as mybir
    import concourse.tile as tile
    from concourse import library_config

    f32 = mybir.dt.float32
    bf16 = mybir.dt.bfloat16
    if mode == "fp16":
        dlo = mybir.dt.float16
        tlo = mybir.dt.float16
    elif mode == "bf16":
        dlo = bf16          # dram dtype of downcast inputs
        tlo = bf16          # SBUF tile dtype for matmul/product operands
    elif mode == "f32r":
        dlo = f32
        tlo = mybir.dt.float32r
    else:
        dlo = f32
        tlo = f32

    def mmin(ap):
        # view a dram AP as the matmul-operand dtype
        return ap if dlo == tlo else ap.bitcast(tlo)

    def vin(ap):
        # view a tile AP as a DVE-operand dtype
        return ap if tlo != mybir.dt.float32r else ap.bitcast(f32)

    P = 128
    Bc = B // n_cores                 # batches per core (16)
    ST = S // P                       # s-tiles per batch (2)
    F = 2 * D                         # concat feature dim (8192)
    DOUT = D // n_cores               # output-dim shard per core (512)
    NC_D = D // 512                   # 512-wide d-chunks for weighted MMs (8)
    NT_W = D // P                     # 128-wide transpose chunks of weighted (32)
    NT_H = D // P                     # 128-wide chunks of hidden (32)
    NKF = F // P                      # 128-wide k-chunks of the linear (64)

    nc = bacc.Bacc(None, target_bir_lowering=False)

    inp = nc.dram_tensor("inp", [S, Bc, D], dlo, kind="ExternalInput")
    hid = nc.dram_tensor("hid", [Bc, D], dlo, kind="ExternalInput")
    hidT = nc.dram_tensor("hidT", [D, B], dlo, kind="ExternalInput")
    wt = nc.dram_tensor("wt", [F, DOUT], dlo, kind="ExternalInput")
    bias = nc.dram_tensor("bias", [1, DOUT], f32, kind="ExternalInput")
    ident = nc.dram_tensor("ident", [P, P], dlo, kind="ExternalInput")
    out = nc.dram_tensor("out", [B, DOUT], f32, kind="ExternalOutput")

    cc_in = nc.dram_tensor("cc_in", [Bc, D], dlo)
    cc_out = nc.dram_tensor("cc_out", [B, D], dlo, addr_space="Shared")

    with tile.TileContext(nc) as tc:
        import contextlib

        with contextlib.ExitStack() as ctx:
            persist = ctx.enter_context(tc.tile_pool(name="persist", bufs=1))

            nc.gpsimd.load_library(library_config.attn)

            ident_sb = persist.tile([P, P], tlo)
            nc.scalar.dma_start(out=ident_sb, in_=mmin(ident[:, :]))

            # masked attn weights: [s, t, b, col]; col b of slice (t, b) is
            # batch b's attn column, everything else stays zero
            attn_diag = persist.tile([P, ST, Bc, Bc], tlo)
            nc.vector.memset(attn_diag[:, :, :, :], 0.0)

            wsum = persist.tile([Bc, D], dlo if mode in ("bf16", "fp16") else f32)

            # linear-stage inputs that stream/land during the batch loop:
            # hidden^T chunks (lhsT of the hidden half) and both W halves,
            # each as one big DMA on queues other than the nat-tile queue.
            hT_sb = persist.tile([P, NT_H, B], tlo)
            nc.scalar.dma_start(
                out=hT_sb,
                in_=mmin(hidT.rearrange("(c p) b -> p c b", p=P)),
            )
            wt_hi = persist.tile([P, NKF // 2, DOUT], tlo)
            nc.scalar.dma_start(
                out=wt_hi,
                in_=mmin(
                    wt[(NKF // 2) * P:, :].rearrange("(c p) d -> p c d", p=P)
                ),
            )
            wt_lo = persist.tile([P, NKF // 2, DOUT], tlo)
            nc.scalar.dma_start(
                out=wt_lo,
                in_=mmin(
                    wt[: (NKF // 2) * P, :].rearrange("(c p) d -> p c d", p=P)
                ),
            )

            # ---------------- attention (batch loop) ----------------
            with contextlib.ExitStack() as loop_ctx:
                natp = loop_ctx.enter_context(tc.tile_pool(name="nat", bufs=3))
                hbp = loop_ctx.enter_context(tc.tile_pool(name="hb", bufs=2))
                prodp = loop_ctx.enter_context(tc.tile_pool(name="prod", bufs=2))
                smalls = loop_ctx.enter_context(tc.tile_pool(name="smalls", bufs=3))
                waccp = loop_ctx.enter_context(
                    tc.tile_pool(name="wacc", bufs=1, space="PSUM")
                )

                wacc = waccp.tile([Bc, D], f32)

                def emit_hb(b):
                    # replicate hid row b across 128 partitions with a single
                    # DMA: free-dim stride-0 source straight from HBM.
                    hb = hbp.tile([P, D], dlo, tag="hb")
                    nc.gpsimd.dma_start(
                        out=hb,
                        in_=hid[b : b + 1, :].unsqueeze(1).to_broadcast(
                            [1, P, D]
                        ),
                    )
                    return hb

                hbs = {0: emit_hb(0)}

                junk = persist.tile([P, D], dlo)

                for b in range(Bc):
                    hb = hbs.pop(b)

                    sc_b = smalls.tile([P, ST], f32, tag="sc")
                    # one 2MB DMA for both s-tiles of this batch
                    nat2 = natp.tile([P, ST, D], tlo, tag="nat")
                    nc.sync.dma_start(
                        out=nat2,
                        in_=mmin(inp[:, b, :].rearrange("(t p) d -> p t d", p=P)),
                    )
                    nats = [nat2[:, t, :] for t in range(ST)]
                    # t=1: plain mult on DVE (2x mode), accumulate on ACT
                    prodB = prodp.tile([P, D], dlo, tag="prodB")
                    nc.vector.tensor_tensor(
                        out=prodB, in0=vin(nats[1]), in1=vin(hb[:, :]),
                        op=mybir.AluOpType.mult,
                    )
                    nc.scalar.activation(
                        out=junk, in_=prodB,
                        func=mybir.ActivationFunctionType.Copy,
                        bias=0.0, scale=1.0,
                        accum_out=sc_b[:, 1:2],
                    )
                    # t=0: fused mult+accum on DVE (no 2x, but single op)
                    prodA = prodp.tile([P, D], dlo, tag="prodA")
                    nc.vector.scalar_tensor_tensor(
                        out=prodA,
                        in0=vin(nats[0]),
                        scalar=1.0,
                        in1=vin(hb[:, :]),
                        op0=mybir.AluOpType.mult,
                        op1=mybir.AluOpType.mult,
                        accum_out=sc_b[:, 0:1],
                    )

                    # prefetch next batch's hidden broadcast
                    if b + 1 < Bc:
                        hbs[b + 1] = emit_hb(b + 1)

                    # softmax over s (partition dim x ST columns)
                    mx2 = smalls.tile([P, ST], f32, tag="mx2")
                    nc.gpsimd.partition_all_reduce(
                        mx2, sc_b, channels=P, reduce_op=bass_isa.ReduceOp.max
                    )
                    negm = smalls.tile([P, 1], f32, tag="negm")
                    nc.vector.tensor_reduce(
                        out=negm, in_=mx2, axis=mybir.AxisListType.X,
                        op=mybir.AluOpType.max, negate=True,
                    )
                    e_b = smalls.tile([P, ST], f32, tag="e_b")
                    s1 = smalls.tile([P, 1], f32, tag="s1")
                    nc.scalar.activation(
                        out=e_b,
                        in_=sc_b,
                        func=mybir.ActivationFunctionType.Exp,
                        bias=negm,
                        scale=1.0,
                        accum_out=s1,
                    )
                    sig = smalls.tile([P, 1], f32, tag="sig")
                    nc.gpsimd.partition_all_reduce(
                        sig, s1, channels=P, reduce_op=bass_isa.ReduceOp.add
                    )
                    r = smalls.tile([P, 1], f32, tag="r")
                    nc.vector.reciprocal(r, sig)
                    attn_b = smalls.tile([P, ST], f32, tag="attn_b")
                    nc.scalar.activation(
                        out=attn_b,
                        in_=e_b,
                        func=mybir.ActivationFunctionType.Copy,
                        bias=0.0,
                        scale=r,
                    )
                    # scatter the two attn columns into their diagonal slots
                    for t in range(ST):
                        nc.scalar.activation(
                            out=attn_diag[:, t, b, b : b + 1],
                            in_=attn_b[:, t : t + 1],
                            func=mybir.ActivationFunctionType.Copy,
                        )

                    # weighted sums: accumulate into wacc rows via masked lhsT
                    for t in range(ST):
                        for c in range(NC_D):
                            nc.tensor.matmul(
                                wacc[:, c * 512 : (c + 1) * 512],
                                attn_diag[:, t, b, :],
                                nats[t][:, c * 512 : (c + 1) * 512],
                                start=(b == 0 and t == 0),
                                stop=(b == Bc - 1 and t == ST - 1),
                            )

                # evacuate weighted PSUM (converts to bf16 in bf16 mode)
                nc.scalar.activation(
                    out=wsum, in_=wacc, func=mybir.ActivationFunctionType.Copy
                )

            # ---------------- exchange ----------------
            nc.sync.dma_start(out=cc_in[:, :], in_=wsum)
            if no_collective:
                for k in range(n_cores):
                    nc.sync.dma_start(
                        out=cc_out[k * Bc : (k + 1) * Bc, :], in_=cc_in[:, :]
                    )
            else:
                nc.gpsimd.collective_compute(
                    "AllGather",
                    mybir.AluOpType.bypass,
                    replica_groups=[list(range(n_cores))],
                    ins=[cc_in[:, :]],
                    outs=[cc_out[:, :]],
                )

            # ---------------- linear ----------------
            with contextlib.ExitStack() as lin_ctx:
                tailp = lin_ctx.enter_context(tc.tile_pool(name="tail", bufs=1))
                wTp = lin_ctx.enter_context(tc.tile_pool(name="wTp", bufs=NT_W))
                tpp = lin_ctx.enter_context(
                    tc.tile_pool(name="tp", bufs=4, space="PSUM")
                )
                linp = lin_ctx.enter_context(
                    tc.tile_pool(name="lin", bufs=1, space="PSUM")
                )

                out_ps = linp.tile([B, DOUT], f32)

                # hidden half first: lhsT chunks come straight from hidT input
                for c in range(NKF // 2):
                    nc.tensor.matmul(
                        out_ps,
                        hT_sb[:, c, :],
                        wt_hi[:, c, :],
                        start=(c == 0),
                        stop=False,
                    )

                # weighted half: gather result, transpose on PE, then matmul
                wag = tailp.tile([B, D], dlo if mode in ("bf16", "fp16") else f32)
                nc.sync.dma_start(out=wag, in_=cc_out[:, :])

                wTs = []
                for c in range(NT_W):
                    tp_ps = tpp.tile([P, B], tlo, tag="tp")
                    nc.tensor.transpose(
                        tp_ps, mmin(wag[:, c * P : (c + 1) * P]),
                        ident_sb[:B, :B],
                    )
                    wT = wTp.tile([P, B], tlo, tag="wT")
                    nc.vector.tensor_copy(wT, tp_ps)
                    wTs.append(wT)

                for c in range(NT_W):
                    nc.tensor.matmul(
                        out_ps,
                        wTs[c],
                        wt_lo[:, c, :],
                        start=False,
                        stop=(c == NT_W - 1),
                    )

                # bias add + store
                bias_sb = tailp.tile([1, DOUT], f32)
                nc.scalar.dma_start(out=bias_sb, in_=bias[:, :])
                bias_bc = tailp.tile([B, DOUT], f32)
                nc.gpsimd.partition_broadcast(bias_bc, bias_sb)
                out_sb = tailp.tile([B, DOUT], f32)
                nc.vector.tensor_add(out_sb, out_ps, bias_bc)
                nc.sync.dma_start(out=out[:, :], in_=out_sb)

    nc.finalize()
    return nc


_CACHE = {}


def _get_program(S, B, D, n_cores):
    key = (S, B, D, n_cores)
    if key not in _CACHE:
        _CACHE[key] = build_program(S, B, D, n_cores)
    return _CACHE[key]


def make_in_maps(inp, hidden, W, b, n_cores=8):
    """Shard host inputs into per-core input maps."""
    S, B, D = inp.shape
    Bc = B // n_cores
    DOUT = W.shape[0] // n_cores
    if MODE == "fp16":
        lo = np.float16
    elif MODE == "bf16":
        import ml_dtypes

        lo = ml_dtypes.bfloat16
    else:
        lo = np.float32
    inp = np.asarray(inp, dtype=np.float32).astype(lo)
    hid0 = np.asarray(hidden[0], dtype=np.float32).astype(lo)   # [B, D]
    hidT = np.ascontiguousarray(hid0.T)                         # [D, B]
    wtT = np.asarray(W, dtype=np.float32).T.astype(lo)          # [F, 4096]
    ident = np.eye(128, dtype=np.float32).astype(lo)
    in_maps = []
    for k in range(n_cores):
        in_maps.append(
            {
                "inp": np.ascontiguousarray(inp[:, k * Bc : (k + 1) * Bc, :]),
                "hid": np.ascontiguousarray(hid0[k * Bc : (k + 1) * Bc, :]),
                "hidT": hidT,
                "wt": np.ascontiguousarray(wtT[:, k * DOUT : (k + 1) * DOUT]),
                "bias": np.ascontiguousarray(
                    np.asarray(b[k * DOUT : (k + 1) * DOUT], dtype=np.float32)
                    .reshape(1, DOUT)
                ),
                "ident": ident,
            }
        )
    return in_maps


def kernel(inp, hidden, W, b, trace=False):
    from concourse.bass_utils import run_bass_kernel_spmd

    inp = np.asarray(inp, dtype=np.float32)
    hidden = np.asarray(hidden, dtype=np.float32)
    W = np.asarray(W, dtype=np.float32)
    b = np.asarray(b, dtype=np.float32)

    S, B, D = inp.shape
    n_cores = 8
    nc = _get_program(S, B, D, n_cores)
    in_maps = make_in_maps(inp, hidden, W, b, n_cores)
    res = run_bass_kernel_spmd(nc, in_maps, core_ids=list(range(n_cores)))
    outs = [res.results[k]["out"] for k in range(n_cores)]
    full = np.concatenate(outs, axis=1)  # [B, D]
    if trace:
        return full[None, :, :], res
    return full[None, :, :]


# revision 9
# speedup vs baseline: 1.3592x; 1.0183x over previous
"""Trainium2 Bass kernel for rank-1 attention + linear (nn_Attention).

Reference computation (S=256, B=128, D=4096):
    scores   = einsum('sbd,bd->bs', inp, hidden[0])      # dot each enc state with hidden
    attn     = softmax(scores, axis=1)                   # over S
    weighted = einsum('bs,sbd->bd', attn, inp)
    concat   = [weighted, hidden[0]]   # [B, 2D]
    out      = concat @ W.T + b        # [1, B, D]

Distribution over 8 NeuronCores:
  - attention part: data-parallel over B (16 batches per core)
  - linear part: W sharded over output dim (512 rows per core); weighted
    vectors exchanged with two on-chip AllGathers (first half mid-loop so
    the collective latency hides under the remaining batches).

All heavy tensors (inp, hidden, W) are downcast to fp16 on the host: halves
HBM traffic and runs PE matmuls at 1 cycle/row (f32 costs 4) while keeping
enough mantissa for the near-tie softmax batches.

Per-core dataflow:
  scores  : per batch, tile 0 = DVE fused scalar_tensor_tensor (mult +
            free-dim accum); tile 1 = DVE tensor_tensor mult (2x mode) with
            the free-dim accumulation on the Scalar engine (Copy+accum).
            hidden row replicated across partitions by a stride-0-source DMA
            (or gpsimd broadcast on every third batch to spread the load).
  softmax : gpsimd partition_all_reduce (max/add) + ACT exp + DVE reciprocal
  weighted: PE matmuls with a column-masked attn matrix (lhsT [s,8], only
            col b%8 nonzero); batches 0-7 accumulate into PSUM rows 0:8,
            batches 8-15 into rows 32:40 so the first half can be evacuated
            and exchanged while the second half still accumulates.
  linear  : host-pretransposed W (f-major, streamed in slabs during the
            loop) + PE transpose of the gathered weighted matrix; 64
            accumulating matmuls + bias.
"""

import sys

if "/opt/trn_rl_repo" not in sys.path:
    sys.path.insert(0, "/opt/trn_rl_repo")

import numpy as np

MODE = "fp16"


# ----------------------------------------------------------------------------
# Program builder
# ----------------------------------------------------------------------------

def build_program(S=256, B=128, D=4096, n_cores=8, no_collective=False,
                  mode=MODE):
    """Build the SPMD Bass program. Returns finalized nc."""
    import concourse.bacc as bacc
    import concourse.bass_isa as bass_isa
    import concourse.mybir as mybir
    import concourse.tile as tile
    from concourse import library_config

    f32 = mybir.dt.float32
    if mode == "fp16":
        dlo = mybir.dt.float16
    elif mode == "bf16":
        dlo = mybir.dt.bfloat16
    else:
        dlo = f32
    tlo = dlo

    P = 128
    Bc = B // n_cores                 # batches per core (16)
    Bh = Bc // 2                      # half-batch group (8)
    ST = S // P                       # s-tiles per batch (2)
    F = 2 * D                         # concat feature dim (8192)
    DOUT = D // n_cores               # output-dim shard per core (512)
    NC_D = D // 512                   # 512-wide d-chunks for weighted MMs (8)
    NT_W = D // P                     # 128-wide transpose chunks of weighted (32)
    NT_H = D // P                     # 128-wide chunks of hidden (32)
    NKF = F // P                      # 128-wide k-chunks of the linear (64)

    nc = bacc.Bacc(None, target_bir_lowering=False)

    inp = nc.dram_tensor("inp", [S, Bc, D], dlo, kind="ExternalInput")
    hid = nc.dram_tensor("hid", [Bc, D], dlo, kind="ExternalInput")
    hidT = nc.dram_tensor("hidT", [D, B], dlo, kind="ExternalInput")
    wt = nc.dram_tensor("wt", [F, DOUT], dlo, kind="ExternalInput")
    bias = nc.dram_tensor("bias", [1, DOUT], f32, kind="ExternalInput")
    ident = nc.dram_tensor("ident", [P, P], dlo, kind="ExternalInput")
    out = nc.dram_tensor("out", [B, DOUT], f32, kind="ExternalOutput")

    cc_inA = nc.dram_tensor("cc_inA", [Bh, D], dlo)
    cc_inB = nc.dram_tensor("cc_inB", [Bh, D], dlo)
    cc_outA = nc.dram_tensor("cc_outA", [B // 2, D], dlo, addr_space="Shared")
    cc_outB = nc.dram_tensor("cc_outB", [B // 2, D], dlo, addr_space="Shared")

    with tile.TileContext(nc) as tc:
        import contextlib

        with contextlib.ExitStack() as ctx:
            persist = ctx.enter_context(tc.tile_pool(name="persist", bufs=1))

            nc.gpsimd.load_library(library_config.attn)

            # small prefetches on the ACT queue (otherwise idle at start)
            ident_sb = persist.tile([P, P], tlo)
            nc.scalar.dma_start(out=ident_sb, in_=ident[:, :])
            bias_sb = persist.tile([1, DOUT], f32)
            nc.scalar.dma_start(out=bias_sb, in_=bias[:, :])
            bias_bc = persist.tile([B, DOUT], f32)
            nc.gpsimd.partition_broadcast(bias_bc, bias_sb)

            # masked attn weights: [s, t, b, col]; col b%8 of slice (t, b) is
            # batch b's attn column, everything else stays zero
            attn_diag = persist.tile([P, ST, Bc, Bh], tlo)
            nc.vector.memset(attn_diag[:, :, :, :], 0.0)

            wsumA = persist.tile([Bh, D], dlo)
            wsumB = persist.tile([Bh, D], dlo)
            wag = persist.tile([B, D], dlo)

            # linear inputs streamed in slabs during the batch loop (on the
            # sync queue, behind each batch's nat tile)
            hT_sb = persist.tile([P, NT_H, B], tlo)
            wt_hi = persist.tile([P, NKF // 2, DOUT], tlo)
            wt_lo = persist.tile([P, NKF // 2, DOUT], tlo)

            def emit_prefetch(b):
                # 4 of the 64 W k-chunks per batch: hidden half (chunks
                # 32..63) during batches 0-7, weighted half during 8-15
                tgt = wt_hi if b < Bh else wt_lo
                base = (NKF // 2) if b < Bh else 0
                c0 = 4 * (b % Bh)
                nc.sync.dma_start(
                    out=tgt[:, c0 : c0 + 4, :],
                    in_=wt[(base + c0) * P : (base + c0 + 4) * P, :]
                    .rearrange("(c p) d -> p c d", p=P),
                )
                # 2 of the 32 hidden^T chunks per batch
                h0 = 2 * b
                nc.sync.dma_start(
                    out=hT_sb[:, h0 : h0 + 2, :],
                    in_=hidT[h0 * P : (h0 + 2) * P, :]
                    .rearrange("(c p) b -> p c b", p=P),
                )

            # ---------------- attention (batch loop) ----------------
            with contextlib.ExitStack() as loop_ctx:
                natp = loop_ctx.enter_context(tc.tile_pool(name="nat", bufs=3))
                hbp = loop_ctx.enter_context(tc.tile_pool(name="hb", bufs=2))
                prodAp = loop_ctx.enter_context(tc.tile_pool(name="prodA", bufs=1))
                prodBp = loop_ctx.enter_context(tc.tile_pool(name="prodB", bufs=2))
                smalls = loop_ctx.enter_context(tc.tile_pool(name="smalls", bufs=8))
                waccp = loop_ctx.enter_context(
                    tc.tile_pool(name="wacc", bufs=1, space="PSUM")
                )

                # rows 0:8 accumulate batches 0-7; rows 32:40 batches 8-15
                # (matmul PSUM outputs must start at partition 0/32/64)
                wacc = waccp.tile([40, D], f32)

                def emit_hb(b):
                    # replicate hid row b across 128 partitions with one DMA:
                    # free-dim stride-0 source straight from HBM
                    hb = hbp.tile([P, D], dlo, tag="hb")
                    nc.gpsimd.dma_start(
                        out=hb,
                        in_=hid[b : b + 1, :].unsqueeze(1).to_broadcast(
                            [1, P, D]
                        ),
                    )
                    return hb

                hbs = {0: emit_hb(0)}

                junk = persist.tile([P, D], dlo)

                for b in range(Bc):
                    hb = hbs.pop(b)
                    g = b // Bh                    # half-group (0 or 1)
                    r0 = 32 * g                    # wacc row base
                    bb = b % Bh                    # column inside the group

                    sc_b = smalls.tile([P, ST], f32, tag="sc")
                    # one 2MB DMA for both s-tiles of this batch
                    nat2 = natp.tile([P, ST, D], tlo, tag="nat")
                    nc.sync.dma_start(
                        out=nat2,
                        in_=inp[:, b, :].rearrange("(t p) d -> p t d", p=P),
                    )
                    emit_prefetch(b)
                    nats = [nat2[:, t, :] for t in range(ST)]
                    # t=1: plain mult on DVE (2x mode), accumulate on ACT
                    prodB = prodBp.tile([P, D], dlo, tag="prodB")
                    nc.vector.tensor_tensor(
                        out=prodB, in0=nats[1], in1=hb[:, :],
                        op=mybir.AluOpType.mult,
                    )
                    nc.scalar.activation(
                        out=junk, in_=prodB,
                        func=mybir.ActivationFunctionType.Copy,
                        bias=0.0, scale=1.0,
                        accum_out=sc_b[:, 1:2],
                    )
                    # t=0: fused mult+accum on DVE
                    prodA = prodAp.tile([P, D], dlo, tag="prodA")
                    nc.vector.scalar_tensor_tensor(
                        out=prodA,
                        in0=nats[0],
                        scalar=1.0,
                        in1=hb[:, :],
                        op0=mybir.AluOpType.mult,
                        op1=mybir.AluOpType.mult,
                        accum_out=sc_b[:, 0:1],
                    )

                    # prefetch the next hidden broadcast
                    if b + 1 < Bc:
                        hbs[b + 1] = emit_hb(b + 1)

                    # softmax over s (partition dim x ST columns)
                    mx2 = smalls.tile([P, ST], f32, tag="mx2")
                    nc.gpsimd.partition_all_reduce(
                        mx2, sc_b, channels=P, reduce_op=bass_isa.ReduceOp.max
                    )
                    negm = smalls.tile([P, 1], f32, tag="negm")
                    nc.vector.tensor_reduce(
                        out=negm, in_=mx2, axis=mybir.AxisListType.X,
                        op=mybir.AluOpType.max, negate=True,
                    )
                    e_b = smalls.tile([P, ST], f32, tag="e_b")
                    s1 = smalls.tile([P, 1], f32, tag="s1")
                    nc.scalar.activation(
                        out=e_b,
                        in_=sc_b,
                        func=mybir.ActivationFunctionType.Exp,
                        bias=negm,
                        scale=1.0,
                        accum_out=s1,
                    )
                    sig = smalls.tile([P, 1], f32, tag="sig")
                    nc.gpsimd.partition_all_reduce(
                        sig, s1, channels=P, reduce_op=bass_isa.ReduceOp.add
                    )
                    r = smalls.tile([P, 1], f32, tag="r")
                    nc.vector.reciprocal(r, sig)
                    attn_b = smalls.tile([P, ST], f32, tag="attn_b")
                    nc.scalar.activation(
                        out=attn_b,
                        in_=e_b,
                        func=mybir.ActivationFunctionType.Copy,
                        bias=0.0,
                        scale=r,
                    )
                    # scatter the two attn columns into their diagonal slots
                    for t in range(ST):
                        nc.scalar.activation(
                            out=attn_diag[:, t, b, bb : bb + 1],
                            in_=attn_b[:, t : t + 1],
                            func=mybir.ActivationFunctionType.Copy,
                        )

                    # weighted sums: accumulate into wacc rows via masked lhsT
                    for t in range(ST):
                        for c in range(NC_D):
                            nc.tensor.matmul(
                                wacc[r0 : r0 + Bh, c * 512 : (c + 1) * 512],
                                attn_diag[:, t, b, :],
                                nats[t][:, c * 512 : (c + 1) * 512],
                                start=(bb == 0 and t == 0),
                                stop=(bb == Bh - 1 and t == ST - 1),
                            )

                    if b == Bh - 1:
                        # first half done: evacuate + exchange while the
                        # second half still accumulates
                        nc.scalar.activation(
                            out=wsumA, in_=wacc[0:Bh, :],
                            func=mybir.ActivationFunctionType.Copy,
                        )
                        nc.sync.dma_start(out=cc_inA[:, :], in_=wsumA)
                        if no_collective:
                            for k in range(n_cores):
                                nc.sync.dma_start(
                                    out=cc_outA[k * Bh : (k + 1) * Bh, :],
                                    in_=cc_inA[:, :],
                                )
                        else:
                            nc.gpsimd.collective_compute(
                                "AllGather",
                                mybir.AluOpType.bypass,
                                replica_groups=[list(range(n_cores))],
                                ins=[cc_inA[:, :]],
                                outs=[cc_outA[:, :]],
                            )
                        for k in range(n_cores):
                            nc.sync.dma_start(
                                out=wag[k * Bc : k * Bc + Bh, :],
                                in_=cc_outA[k * Bh : (k + 1) * Bh, :],
                            )

                # second half: evacuate + exchange
                nc.scalar.activation(
                    out=wsumB, in_=wacc[32 : 32 + Bh, :],
                    func=mybir.ActivationFunctionType.Copy,
                )
                nc.sync.dma_start(out=cc_inB[:, :], in_=wsumB)
                if no_collective:
                    for k in range(n_cores):
                        nc.sync.dma_start(
                            out=cc_outB[k * Bh : (k + 1) * Bh, :],
                            in_=cc_inB[:, :],
                        )
                else:
                    nc.gpsimd.collective_compute(
                        "AllGather",
                        mybir.AluOpType.bypass,
                        replica_groups=[list(range(n_cores))],
                        ins=[cc_inB[:, :]],
                        outs=[cc_outB[:, :]],
                    )
                for k in range(n_cores):
                    nc.sync.dma_start(
                        out=wag[k * Bc + Bh : (k + 1) * Bc, :],
                        in_=cc_outB[k * Bh : (k + 1) * Bh, :],
                    )

            # ---------------- linear ----------------
            with contextlib.ExitStack() as lin_ctx:
                tailp = lin_ctx.enter_context(tc.tile_pool(name="tail", bufs=1))
                wTp = lin_ctx.enter_context(tc.tile_pool(name="wTp", bufs=NT_W))
                tpp = lin_ctx.enter_context(
                    tc.tile_pool(name="tp", bufs=4, space="PSUM")
                )
                linp = lin_ctx.enter_context(
                    tc.tile_pool(name="lin", bufs=1, space="PSUM")
                )

                out_ps = linp.tile([B, DOUT], f32)

                # hidden half first: lhsT chunks from the prefetched hidT
                for c in range(NKF // 2):
                    nc.tensor.matmul(
                        out_ps,
                        hT_sb[:, c, :],
                        wt_hi[:, c, :],
                        start=(c == 0),
                        stop=False,
                    )

                # weighted half: transpose the gathered matrix on PE
                wTs = []
                for c in range(NT_W):
                    tp_ps = tpp.tile([P, B], tlo, tag="tp")
                    nc.tensor.transpose(
                        tp_ps, wag[:, c * P : (c + 1) * P], ident_sb[:B, :B]
                    )
                    wT = wTp.tile([P, B], tlo, tag="wT")
                    nc.vector.tensor_copy(wT, tp_ps)
                    wTs.append(wT)

                for c in range(NT_W):
                    nc.tensor.matmul(
                        out_ps,
                        wTs[c],
                        wt_lo[:, c, :],
                        start=False,
                        stop=(c == NT_W - 1),
                    )

                # bias add + store
                out_sb = tailp.tile([B, DOUT], f32)
                nc.vector.tensor_add(out_sb, out_ps, bias_bc)
                nc.sync.dma_start(out=out[:, :], in_=out_sb)

    nc.finalize()
    return nc


_CACHE = {}


def _get_program(S, B, D, n_cores):
    key = (S, B, D, n_cores)
    if key not in _CACHE:
        _CACHE[key] = build_program(S, B, D, n_cores)
    return _CACHE[key]


def make_in_maps(inp, hidden, W, b, n_cores=8):
    """Shard host inputs into per-core input maps."""
    S, B, D = inp.shape
    Bc = B // n_cores
    DOUT = W.shape[0] // n_cores
    if MODE == "fp16":
        lo = np.float16
    elif MODE == "bf16":
        import ml_dtypes

        lo = ml_dtypes.bfloat16
    else:
        lo = np.float32
    inp = np.asarray(inp, dtype=np.float32).astype(lo)
    hid0 = np.asarray(hidden[0], dtype=np.float32).astype(lo)   # [B, D]
    hidT = np.ascontiguousarray(hid0.T)                         # [D, B]
    wtT = np.asarray(W, dtype=np.float32).T.astype(lo)          # [F, 4096]
    ident = np.eye(128, dtype=np.float32).astype(lo)
    in_maps = []
    for k in range(n_cores):
        in_maps.append(
            {
                "inp": np.ascontiguousarray(inp[:, k * Bc : (k + 1) * Bc, :]),
                "hid": np.ascontiguousarray(hid0[k * Bc : (k + 1) * Bc, :]),
                "hidT": hidT,
                "wt": np.ascontiguousarray(wtT[:, k * DOUT : (k + 1) * DOUT]),
                "bias": np.ascontiguousarray(
                    np.asarray(b[k * DOUT : (k + 1) * DOUT], dtype=np.float32)
                    .reshape(1, DOUT)
                ),
                "ident": ident,
            }
        )
    return in_maps


def kernel(inp, hidden, W, b, trace=False):
    from concourse.bass_utils import run_bass_kernel_spmd

    inp = np.asarray(inp, dtype=np.float32)
    hidden = np.asarray(hidden, dtype=np.float32)
    W = np.asarray(W, dtype=np.float32)
    b = np.asarray(b, dtype=np.float32)

    S, B, D = inp.shape
    n_cores = 8
    nc = _get_program(S, B, D, n_cores)
    in_maps = make_in_maps(inp, hidden, W, b, n_cores)
    res = run_bass_kernel_spmd(nc, in_maps, core_ids=list(range(n_cores)))
    outs = [res.results[k]["out"] for k in range(n_cores)]
    full = np.concatenate(outs, axis=1)  # [B, D]
    if trace:
        return full[None, :, :], res
    return full[None, :, :]
